# revision 34
# baseline (speedup 1.0000x reference)
"""Trainium2 Bass kernel for nn_DiffusionCNN (submanifold sparse 3x3x3 CNN).

Strategy (8-core SPMD, no collectives):
  - Shard the voxel dim N=200000 into 8 contiguous blocks of 25000 (voxels are
    sorted by linear grid index, so neighbor indices are within ~+-1800 rows).
    Halo compute is replicated instead of exchanged.
  - Only ~13% of the 27 neighbor offsets are valid (occupancy ~9.5%); the
    baseline gathered all 27 (mostly zero rows).  Here we gather ONLY valid
    neighbors, compacted per 512-row tile and grouped by offset k:
      * per-offset z-matmuls: z[j,:] = W_k^T x[nbr_j]  (PE, 32/64/128-wide
        slots at aligned PSUM partition offsets),
      * expansion matmuls with a 0/1 matrix A[j,i] = (seg[j]==i) built on
        the DVE via iota/is_equal, accumulated into the [128,512] output.
    The center offset (always valid, identity map) skips the gather: its
    term is a single matmul against a channel-major resident copy of x/h1.
  - Slot sizes per (tile,k) are the max over the 8 cores, so ONE compiled
    program serves all cores; per-core index/seg tables fill the slots.
  - Gathers stay on one SWDGE queue: concurrent gathers on different queues
    corrupt data on this stack (verified with a minimal repro).
  - All matmuls in bf16 with fp32 PSUM accumulation.

Host-side work is limited to sharding/marshalling: slicing inputs, remapping
neighbor indices to per-core compacted tables, packing weights, re-assembling
the output.
"""

import os
import numpy as np
import ml_dtypes

# ---------------------------------------------------------------- constants
N = 200000
PER = 25000
NCORES = 8
C = 128
K = 27
CENTER = 13
TEMB = 6
IN_CH = 7  # features(1) + sin/cos(6)

TILE = 512
NT1 = 56                 # h1 tiles per core
NT2 = 49                 # output tiles per core
M_H1 = NT1 * TILE        # 28672 h1 rows computed per core (incl. halo + pad)
M_OUT = NT2 * TILE       # 25088 output rows per core (25000 + pad)
NZ = 512                 # zero rows at the front of each gather table
H1T = NZ + M_H1          # h1 table rows
# NOTE: concurrent gathers on different SWDGE queues corrupt data on this
# stack (verified with a minimal repro) -- stay on one queue.
NQ = int(os.environ.get("KNQ", "1"))   # SWDGE queues declared
NQ_USE = int(os.environ.get("KNQ_USE", str(NQ)))  # queues actually used

_bf16 = ml_dtypes.bfloat16


# ------------------------------------------------------------------ layout
def _pack_tile_items(items):
    """items: list of (k, n) with exact n in [1,128].  Pack into 128-row PSUM
    chunks.  PSUM partition bases are limited to {0,32,64}: each chunk holds
    either one slot with n>64 (base 0), or up to two slots with n<=64 at
    bases 0 and 64.  Gather columns pack tightly (gcol assigned later).
    Returns list of chunks; each chunk is a list of (k, jb, n) slots."""
    big = [(k, n) for (k, n) in items if n > 64]
    mid = [(k, n) for (k, n) in items if 32 < n <= 64]
    small = [(k, n) for (k, n) in items if n <= 32]
    chunks = [[(k, 0, n)] for (k, n) in big]
    while len(mid) >= 2:
        a, b = mid.pop(), mid.pop()
        chunks.append([(a[0], 0, a[1]), (b[0], 64, b[1])])
    if mid:
        # one leftover mid: put it at 64, fill 0/32 with smalls if any
        a = mid.pop()
        ents = []
        if small:
            s = small.pop()
            ents.append((s[0], 0, s[1]))
        if small:
            s = small.pop()
            ents.append((s[0], 32, s[1]))
        ents.append((a[0], 64, a[1]))
        chunks.append(ents)
    while small:
        # up to three smalls per chunk at bases 0/32/64 (64-base may be <=64)
        ents = []
        for base in (0, 32, 64):
            if small:
                s = small.pop()
                ents.append((s[0], base, s[1]))
        chunks.append(ents)
    if not chunks:
        chunks.append([(-1, 0, 1)])
    return chunks


def _build_layout(counts, NT):
    """counts: [NCORES][NT][K] valid counts.  Returns per-tile chunk layouts
    shared by all cores (slot budget = exact max over cores)."""
    tiles = []
    for t in range(NT):
        items = []
        for k in range(K):
            if k == CENTER:
                continue
            B = max(counts[c][t][k] for c in range(NCORES))
            if B == 0:
                continue
            assert B <= 128, (t, k, B)
            items.append((k, B))
        tiles.append(_pack_tile_items(items))
    return tiles


def _layout_cols(chunks):
    """Assign packed gather-column offsets to each slot.  Returns
    (slots, mt): slots = [(k, chunk, jb, n, gcol)], mt = padded idx count."""
    slots = []
    col = 0
    for cix, ents in enumerate(chunks):
        for (k, jb, n) in ents:
            slots.append((k, cix, jb, n, col))
            col += n
    mt = (col + 127) // 128 * 128
    return slots, mt


# ------------------------------------------------------------- device program
def _build_program(layout1, layout2, halo1, xt, bench_reps=0):
    import concourse.mybir as mybir
    import concourse.tile as tile
    from concourse import bacc
    from concourse.masks import make_identity

    bf = mybir.dt.bfloat16
    f32 = mybir.dt.float32
    i16 = mybir.dt.int16
    i32 = mybir.dt.int32
    AF = mybir.ActivationFunctionType
    EQ = mybir.AluOpType.is_equal

    maxch = max(len(t) for t in (layout1 + layout2))
    cols1 = [_layout_cols(t) for t in layout1]
    cols2 = [_layout_cols(t) for t in layout2]
    mts1 = [mt for (_, mt) in cols1]
    mts2 = [mt for (_, mt) in cols2]
    maxm = max(mts1 + mts2)
    ilen1, ilen2 = sum(mts1) // 16, sum(mts2) // 16
    slen1 = sum(len(t) for t in layout1)
    slen2 = sum(len(t) for t in layout2)

    nc = bacc.Bacc("TRN2", target_bir_lowering=False, debug=False,
                   num_swdge_queues=NQ)

    x_tab = nc.dram_tensor("x_tab", [xt, C], bf, kind="ExternalInput")
    xcm = nc.dram_tensor("xcm", [8, M_H1], bf, kind="ExternalInput")
    i1 = nc.dram_tensor("i1", [128, ilen1], i16, kind="ExternalInput")
    i2 = nc.dram_tensor("i2", [128, ilen2], i16, kind="ExternalInput")
    s1 = nc.dram_tensor("s1", [128, slen1], f32, kind="ExternalInput")
    s2 = nc.dram_tensor("s2", [128, slen2], f32, kind="ExternalInput")
    w1 = nc.dram_tensor("w1", [C, K * C], bf, kind="ExternalInput")
    w1c = nc.dram_tensor("w1c", [8, C], bf, kind="ExternalInput")
    w2 = nc.dram_tensor("w2", [C, K * C], bf, kind="ExternalInput")
    w3 = nc.dram_tensor("w3", [C, C], bf, kind="ExternalInput")
    w4 = nc.dram_tensor("w4", [C, 16], bf, kind="ExternalInput")
    b1 = nc.dram_tensor("b1", [C, 1], f32, kind="ExternalInput")
    b2 = nc.dram_tensor("b2", [C, 1], f32, kind="ExternalInput")
    b3 = nc.dram_tensor("b3", [C, 1], f32, kind="ExternalInput")
    b4 = nc.dram_tensor("b4", [1, 1], f32, kind="ExternalInput")
    outd = nc.dram_tensor("out", [NT2, TILE], f32, kind="ExternalOutput")
    h1tab = nc.dram_tensor("h1_tab", [H1T, C], bf, kind="Internal")

    with tile.TileContext(nc) as tc:
        with (
            tc.tile_pool(name="const", bufs=1) as constp,
            tc.tile_pool(name="idx", bufs=4) as idxp,
            tc.tile_pool(name="seg", bufs=3) as segp,
            tc.tile_pool(name="gat", bufs=4) as gatp,
            tc.tile_pool(name="amat", bufs=2) as apool,
            tc.tile_pool(name="zsb", bufs=2) as zpool,
            tc.tile_pool(name="act", bufs=3) as actp,
            tc.tile_pool(name="stage", bufs=3) as stagep,
            tc.tile_pool(name="ost", bufs=3) as outp,
            tc.tile_pool(name="psz", bufs=2, space="PSUM") as pszp,
            tc.tile_pool(name="psacc", bufs=2, space="PSUM") as psacc,
            tc.tile_pool(name="pstr", bufs=2, space="PSUM") as pstr,
            tc.tile_pool(name="psout", bufs=2, space="PSUM") as psout,
        ):
            w1_sb = constp.tile([C, K * C], bf, tag="w1")
            nc.sync.dma_start(w1_sb[:], w1[:])
            w1c_sb = constp.tile([8, C], bf, tag="w1c")
            nc.sync.dma_start(w1c_sb[:], w1c[:])
            w2_sb = constp.tile([C, K * C], bf, tag="w2")
            nc.sync.dma_start(w2_sb[:], w2[:])
            w3_sb = constp.tile([C, C], bf, tag="w3")
            nc.sync.dma_start(w3_sb[:], w3[:])
            w4_sb = constp.tile([C, 16], bf, tag="w4")
            nc.sync.dma_start(w4_sb[:], w4[:])
            b1_sb = constp.tile([C, 1], f32, tag="b1")
            nc.sync.dma_start(b1_sb[:], b1[:])
            b2_sb = constp.tile([C, 1], f32, tag="b2")
            nc.sync.dma_start(b2_sb[:], b2[:])
            b3_sb = constp.tile([C, 1], f32, tag="b3")
            nc.sync.dma_start(b3_sb[:], b3[:])
            b4_sb = constp.tile([1, 1], f32, tag="b4")
            nc.sync.dma_start(b4_sb[:], b4[:])
            bz_sb = constp.tile([C, 1], f32, tag="bz")
            nc.vector.memset(bz_sb[:], 0.0)
            ident = constp.tile([C, C], bf, tag="ident")
            make_identity(nc, ident[:])
            iota_i = constp.tile([128, TILE], i32, tag="iotai")
            nc.gpsimd.iota(iota_i[:], pattern=[[1, TILE]],
                           channel_multiplier=0)
            iota_sb = constp.tile([128, TILE], f32, tag="iota")
            nc.vector.tensor_copy(iota_sb[:], iota_i[:])
            xcm_sb = constp.tile([8, M_H1], bf, tag="xcm")
            nc.sync.dma_start(xcm_sb[:], xcm[:])
            h1cm = constp.tile([C, M_H1], bf, tag="h1cm")
            zblk = constp.tile([C, NZ // 128 * C], bf, tag="zblk")
            nc.vector.memset(zblk[:], 0.0)
            nc.sync.dma_start(
                h1tab[0:NZ, :].rearrange("(p c) e -> p (c e)", c=NZ // 128),
                zblk[:])

            # init the z PSUM buffers so stale slot gaps are finite
            for zi in range(2):
                zps0 = pszp.tile([128, 512], f32, tag="zp")
                nc.vector.memset(zps0[:], 0.0)

            nchunk = TILE // 128
            ioff = {1: 0, 2: 0}
            soff = {1: 0, 2: 0}
            qctr = [0]
            reg_cache = {}

            def conv_tile(ph, t, chunks):
                nch = len(chunks)
                slots, mt = _layout_cols(chunks)
                iN, sN = (i1, s1) if ph == 1 else (i2, s2)
                tab = x_tab if ph == 1 else h1tab
                w_sb = w1_sb if ph == 1 else w2_sb
                io, so = ioff[ph], soff[ph]
                ioff[ph] += mt // 16
                soff[ph] += nch

                it = idxp.tile([128, maxm // 16], i16, tag="it")
                nc.sync.dma_start(it[:, :mt // 16],
                                  iN[:, io:io + mt // 16])
                sg = segp.tile([128, maxch], f32, tag="sg")
                nc.sync.dma_start(sg[:, :nch], sN[:, so:so + nch])
                g = gatp.tile([128, maxm], bf, tag="g")
                if mt not in reg_cache:
                    reg_cache[mt] = nc.gpsimd.to_reg(mt)
                nc.gpsimd.dma_gather(
                    out_ap=g[:, :mt].rearrange("p (o n) -> p o n", o=1),
                    in_ap=tab[:, :],
                    idxs_ap=it[:, :mt // 16],
                    num_idxs=mt,
                    num_idxs_reg=reg_cache[mt],
                    elem_size=C,
                    transpose=True,
                    single_packet=False,
                    queue_num=qctr[0] % NQ_USE,
                )
                qctr[0] += 1

                # expansion matrices A[j, i] = (seg[j] == i)
                A = apool.tile([128, maxch * TILE], bf, tag="A")
                for cix in range(nch):
                    nc.vector.tensor_scalar(
                        A[:, cix * TILE:(cix + 1) * TILE],
                        iota_sb[:], sg[:, cix:cix + 1], None, op0=EQ)

                # z[j, :] = W_k^T g[:, j], packed 4 chunks per PSUM bank
                zsb = zpool.tile([128, maxch * 128], bf, tag="z")
                for g0 in range(0, nch, 4):
                    gw = min(4, nch - g0)
                    zps = pszp.tile([128, 512], f32, tag="zp")
                    for (k, cix, jb, sz, gcol) in slots:
                        if not (g0 <= cix < g0 + gw):
                            continue
                        kk = 0 if k < 0 else k
                        nc.tensor.matmul(
                            zps[jb:jb + sz,
                                (cix - g0) * 128:(cix - g0 + 1) * 128],
                            lhsT=g[:, gcol:gcol + sz],
                            rhs=w_sb[:, kk * C:(kk + 1) * C],
                            start=True, stop=True)
                    nc.scalar.activation(
                        zsb[:, g0 * 128:(g0 + gw) * 128],
                        zps[:, :gw * 128], AF.Identity, bias=bz_sb[:, 0:1])

                acc = psacc.tile([C, TILE], f32, tag="acc")
                if ph == 1:
                    nc.tensor.matmul(
                        acc[:], lhsT=w1c_sb[:],
                        rhs=xcm_sb[0:8, t * TILE:(t + 1) * TILE],
                        start=True, stop=False)
                else:
                    nc.tensor.matmul(
                        acc[:], lhsT=w2_sb[:, CENTER * C:(CENTER + 1) * C],
                        rhs=h1cm[:, halo1 + t * TILE:halo1 + (t + 1) * TILE],
                        start=True, stop=False)
                for cix in range(nch):
                    nc.tensor.matmul(
                        acc[:], lhsT=zsb[:, cix * 128:(cix + 1) * 128],
                        rhs=A[:, cix * TILE:(cix + 1) * TILE],
                        start=False, stop=(cix == nch - 1))
                return acc

            def emit_body():
                ioff[1] = ioff[2] = soff[1] = soff[2] = 0
                # ------------ phase 1: h1 = silu(conv1(x)) ------------------
                for t in range(NT1):
                    acc = conv_tile(1, t, layout1[t])
                    nc.scalar.activation(
                        h1cm[:, t * TILE:(t + 1) * TILE], acc[:],
                        AF.Silu, bias=b1_sb[:, 0:1])
                    pt = pstr.tile([C, TILE], bf, tag="tr")
                    for cch in range(nchunk):
                        nc.tensor.matmul(
                            pt[:, 128 * cch:128 * (cch + 1)],
                            lhsT=h1cm[:, t * TILE + 128 * cch:
                                      t * TILE + 128 * (cch + 1)],
                            rhs=ident[:],
                            is_transpose=True,
                            start=(cch == 0),
                            stop=(cch == nchunk - 1),
                        )
                    st = stagep.tile([C, TILE], bf, tag="st")
                    nc.vector.tensor_copy(st[:], pt[:])
                    r0 = NZ + t * TILE
                    # physical row of logical in-tile row r is
                    # 4*(r%128) + r//128 (host compensates in conv2 indices)
                    nc.sync.dma_start(
                        h1tab[r0:r0 + TILE, :].rearrange(
                            "(p c) e -> p (c e)", c=nchunk),
                        st[:],
                    )

                # ------------ phase 2: conv2 + pointwise MLP ---------------
                for t in range(NT2):
                    acc = conv_tile(2, t, layout2[t])
                    h2 = actp.tile([C, TILE], bf, tag="h")
                    nc.scalar.activation(h2[:], acc[:], AF.Silu,
                                         bias=b2_sb[:, 0:1])
                    ps3 = psacc.tile([C, TILE], f32, tag="acc")
                    nc.tensor.matmul(ps3[:], lhsT=w3_sb[:], rhs=h2[:],
                                     start=True, stop=True)
                    h3 = actp.tile([C, TILE], bf, tag="h")
                    nc.scalar.activation(h3[:], ps3[:], AF.Silu,
                                         bias=b3_sb[:, 0:1])
                    ps4 = psout.tile([1, TILE], f32, tag="o")
                    nc.tensor.matmul(ps4[:], lhsT=w4_sb[:, 0:1], rhs=h3[:],
                                     start=True, stop=True)
                    ost = outp.tile([1, TILE], f32, tag="ost")
                    nc.scalar.activation(ost[0:1, :], ps4[:], AF.Identity,
                                         bias=b4_sb[0:1, 0:1])
                    nc.sync.dma_start(outd[t:t + 1, :], ost[0:1, :])

            if bench_reps > 0:
                with tc.For_i(0, bench_reps, 1):
                    emit_body()
            else:
                emit_body()

    nc.compile()
    return nc


# ------------------------------------------------------------------ host prep
def _sinusoidal(t):
    half = TEMB // 2
    freqs = (np.float32(2.0) ** np.arange(half, dtype=np.float32)) \
        * np.float32(np.pi)
    ang = t.astype(np.float32)[:, None] * freqs[None, :]
    return np.concatenate([np.sin(ang), np.cos(ang)], -1).astype(np.float32)


def _wrap_idx(flat):
    """[M] int32 -> [128, M/16] int16 (16-partition wrap, replicated x8)."""
    m = flat.shape[0]
    a = flat.reshape(m // 16, 16).T
    return np.tile(a, (8, 1)).astype(np.int16)


def _phys_h1_row(j):
    t = j // TILE
    r = j % TILE
    return t * TILE + 4 * (r % 128) + r // 128


def _core_valid(core, nidx, halo1):
    """Per-core valid lists for both phases.

    Returns (vals1, vals2, counts1, counts2, lo1) where valsP[t][k] =
    (pos_array, seg_array); pos is the global voxel row of the neighbor."""
    s = core * PER
    lo1 = s - halo1

    jl = np.arange(M_H1, dtype=np.int64)
    gj = lo1 + jl
    inb = (gj >= 0) & (gj < N)
    sub1 = np.full((K, M_H1), N, np.int32)
    sub1[:, inb] = nidx[:, gj[inb]]

    sub2 = np.full((K, M_OUT), N, np.int32)
    sub2[:, :PER] = nidx[:, s:s + PER]

    msk1 = np.stack([(sub1[12] < N), (sub1[14] < N)]).astype(_bf16)
    msk2 = np.stack([(sub2[12] < N), (sub2[14] < N)]).astype(_bf16)

    def per_tile(sub, NT):
        vals, counts = [], []
        for t in range(NT):
            blk = sub[:, t * TILE:(t + 1) * TILE]
            v = {}
            cnt = np.zeros(K, np.int32)
            for k in range(K):
                if k == CENTER:
                    continue
                segs = np.nonzero(blk[k] < N)[0]
                if len(segs) == 0:
                    continue
                v[k] = (blk[k][segs].astype(np.int64), segs.astype(np.int32))
                cnt[k] = len(segs)
            vals.append(v)
            counts.append(cnt)
        return vals, counts

    vals1, counts1 = per_tile(sub1, NT1)
    vals2, counts2 = per_tile(sub2, NT2)
    return vals1, vals2, counts1, counts2, lo1, msk1, msk2


def _fill_core(layout, vals, to_idx, rng):
    """Build the per-core gather-index and seg tables for one phase.

    Gather columns are packed (slot gcol offsets); seg rows are indexed by
    PSUM (chunk, partition)."""
    gidx_parts, gseg_parts = [], []
    for t, chunks in enumerate(layout):
        nch = len(chunks)
        slots, mt = _layout_cols(chunks)
        gi = rng.integers(0, NZ, size=mt).astype(np.int32)
        gs = np.full(nch * 128, -1, np.int32)
        for (k, cix, jb, sz, gcol) in slots:
            if k < 0 or k not in vals[t]:
                continue
            pos_arr, seg_arr = vals[t][k]
            idxs = to_idx(pos_arr)
            n = len(idxs)
            assert n <= sz, (t, k, n, sz)
            # ring r serves flat columns == r (mod 16); give each ring an
            # ascending address walk (measured ~13% faster than stride-16
            # sampling of the ascending run)
            perm = np.argsort((gcol + np.arange(n)) % 16, kind="stable")
            byp = np.argsort(idxs, kind="stable")
            gi[gcol + perm] = idxs[byp]
            gs[cix * 128 + jb + perm] = seg_arr[byp]
        gidx_parts.append(gi)
        gseg_parts.append(gs.reshape(nch, 128).T)
    gidx = np.concatenate(gidx_parts)
    gseg = np.concatenate(gseg_parts, axis=1)
    assert gidx.max() < 32768
    return _wrap_idx(gidx), gseg.astype(np.float32)


_PREP_CACHE = {}


def _prep_all(inputs):
    key = id(inputs.get("neighbor_idx"))
    features = np.asarray(inputs["features"], np.float32)
    t = np.asarray(inputs["t"])
    nidx = np.asarray(inputs["neighbor_idx"]).astype(np.int32)
    x_full = np.concatenate([features, _sinusoidal(t)], -1)

    # fixed h1 halo so the h1 window offset is core-independent
    halos, posts = [], []
    for core in range(NCORES):
        s, e = core * PER, (core + 1) * PER
        v = nidx[:, s:e]
        v = v[v < N]
        halos.append(max(0, s - int(v.min())))
        posts.append(max(0, int(v.max()) + 1 - e))
    halo1 = max(halos)
    post1 = max(posts)
    assert halo1 + PER + post1 <= M_H1, (halo1, post1)
    assert halo1 + M_OUT <= M_H1, halo1

    cores = []
    for core in range(NCORES):
        (vals1, vals2, counts1, counts2, lo1,
         msk1, msk2) = _core_valid(core, nidx, halo1)
        cores.append(dict(vals1=vals1, vals2=vals2, counts1=counts1,
                          counts2=counts2, lo1=lo1, msk1=msk1, msk2=msk2))

    layout1 = _build_layout([c["counts1"] for c in cores], NT1)
    layout2 = _build_layout([c["counts2"] for c in cores], NT2)

    # shared weights
    W1 = np.asarray(inputs["W1"], np.float32)
    W2 = np.asarray(inputs["W2"], np.float32)
    W1p = np.zeros((K, C, C), np.float32)
    W1p[:, :IN_CH, :] = W1
    w1d = np.ascontiguousarray(
        W1p.transpose(1, 0, 2).reshape(C, K * C)).astype(_bf16)
    w1cd = np.zeros((8, C), _bf16)
    w1cd[:IN_CH] = W1[CENTER].astype(_bf16)
    w2d = np.ascontiguousarray(
        W2.transpose(1, 0, 2).reshape(C, K * C)).astype(_bf16)
    w3d = np.ascontiguousarray(np.asarray(inputs["W3"], np.float32)) \
        .astype(_bf16)
    w4d = np.zeros((C, 16), _bf16)
    w4d[:, 0] = np.asarray(inputs["W4"], np.float32)[:, 0].astype(_bf16)
    shared = {
        "w1": w1d, "w1c": w1cd, "w2": w2d, "w3": w3d, "w4": w4d,
        "b1": np.asarray(inputs["b1"], np.float32).reshape(C, 1).copy(),
        "b2": np.asarray(inputs["b2"], np.float32).reshape(C, 1).copy(),
        "b3": np.asarray(inputs["b3"], np.float32).reshape(C, 1).copy(),
        "b4": np.asarray(inputs["b4"], np.float32).reshape(1, 1).copy(),
    }

    # x windows (per core), then a shared table size
    for cd in cores:
        allpos = [v[0] for vt in cd["vals1"] for v in vt.values()]
        allpos = np.concatenate(allpos) if allpos else np.array([0])
        cd["lo0"] = int(allpos.min())
        cd["hi0"] = int(allpos.max() + 1)
    xt = NZ + max(cd["hi0"] - cd["lo0"] for cd in cores)
    xt = (xt + 127) // 128 * 128
    assert xt < 32768, xt

    in_maps = []
    for core in range(NCORES):
        cd = cores[core]
        lo1 = cd["lo1"]
        lo0, hi0 = cd["lo0"], cd["hi0"]
        n0 = hi0 - lo0

        x_tab = np.zeros((xt, C), _bf16)
        x_tab[NZ:NZ + n0, :IN_CH] = x_full[lo0:hi0].astype(_bf16)

        xcm = np.zeros((8, M_H1), _bf16)
        jl = np.arange(M_H1, dtype=np.int64)
        gj = lo1 + jl
        inb = (gj >= 0) & (gj < N)
        xcm[:IN_CH, inb] = x_full[gj[inb]].astype(_bf16).T

        rng = np.random.default_rng(12345 + core)
        i1d, s1d = _fill_core(layout1, cd["vals1"],
                              lambda p: p - lo0 + NZ, rng)
        i2d, s2d = _fill_core(layout2, cd["vals2"],
                              lambda p: NZ + _phys_h1_row(p - lo1), rng)
        m = {"x_tab": x_tab, "xcm": xcm,
             "i1": i1d, "s1": s1d, "i2": i2d, "s2": s2d}
        m.update(shared)
        in_maps.append(m)

    return in_maps, layout1, layout2, halo1, xt


# ------------------------------------------------------------------ execution
def _run_pjrt(nc, in_maps, reps=0):
    """Execute the Bass program on the 8 axon-tunneled cores via PJRT."""
    import time as _time
    import jax
    from jax.sharding import Mesh, NamedSharding, PartitionSpec
    from jax.experimental.shard_map import shard_map
    import concourse.mybir as mybir
    from concourse import bass2jax

    bass2jax.install_neuronx_cc_hook()

    n_cores = len(in_maps)
    partition_name = (
        nc.partition_id_tensor.name if nc.partition_id_tensor else None
    )
    in_names, out_names, out_avals, zero_outs = [], [], [], []
    for alloc in nc.m.functions[0].allocations:
        if not isinstance(alloc, mybir.MemoryLocationSet):
            continue
        name = alloc.memorylocations[0].name
        if alloc.kind == "ExternalInput":
            if name != partition_name:
                in_names.append(name)
        elif alloc.kind == "ExternalOutput":
            shape = tuple(alloc.tensor_shape)
            dtype = mybir.dt.np(alloc.dtype)
            out_names.append(name)
            out_avals.append(jax.core.ShapedArray(shape, dtype))
            zero_outs.append(np.zeros(shape, dtype))
    n_params = len(in_names)
    n_outs = len(out_names)
    all_names = in_names + out_names
    if partition_name is not None:
        all_names = all_names + [partition_name]
    donate = tuple(range(n_params, n_params + n_outs))

    def _body(*args):
        operands = list(args)
        if partition_name is not None:
            operands.append(bass2jax.partition_id_tensor())
        outs = bass2jax._bass_exec_p.bind(
            *operands,
            out_avals=tuple(out_avals),
            in_names=tuple(all_names),
            out_names=tuple(out_names),
            lowering_input_output_aliases=(),
            sim_require_finite=True,
            sim_require_nnan=True,
            nc=nc,
        )
        return tuple(outs)

    devices = jax.devices()[:n_cores]
    mesh = Mesh(np.asarray(devices), ("core",))
    spec = PartitionSpec("core")
    sharded = jax.jit(
        shard_map(_body, mesh=mesh, in_specs=(spec,) * (n_params + n_outs),
                  out_specs=(spec,) * n_outs, check_rep=False),
        donate_argnums=donate,
        keep_unused=True,
    )
    concat_in = [
        np.concatenate([np.asarray(m[name]) for m in in_maps], axis=0)
        for name in in_names
    ]
    sh = NamedSharding(mesh, spec)
    inp_dev = [jax.device_put(a, sh) for a in concat_in]

    def _zeros():
        return [np.zeros((n_cores * z.shape[0], *z.shape[1:]), z.dtype)
                for z in zero_outs]

    out_arrs = sharded(*inp_dev, *_zeros())
    jax.block_until_ready(out_arrs)
    results = [
        {name: np.asarray(out_arrs[i]).reshape(n_cores, *out_avals[i].shape)[c]
         for i, name in enumerate(out_names)}
        for c in range(n_cores)
    ]

    times = []
    for _ in range(reps):
        zs = _zeros()
        t0 = _time.perf_counter()
        o = sharded(*inp_dev, *zs)
        jax.block_until_ready(o)
        times.append(_time.perf_counter() - t0)
    return results, times


_NC_CACHE = {}


def _run(inputs, reps=0):
    in_maps, layout1, layout2, halo1, xt = _prep_all(inputs)
    if "nc" not in _NC_CACHE:
        _NC_CACHE["nc"] = _build_program(layout1, layout2, halo1, xt)
    nc = _NC_CACHE["nc"]
    results, times = _run_pjrt(nc, in_maps, reps=reps)
    out = np.empty((N, 1), np.float32)
    for core in range(NCORES):
        out[core * PER:(core + 1) * PER, 0] = \
            results[core]["out"].reshape(M_OUT)[:PER]
    return out, times


def kernel(**inputs) -> np.ndarray:
    out, _ = _run(inputs, reps=0)
    return out


def bench(inputs, loop_reps=(1, 26), wall_reps=8):
    """Estimate on-device kernel time by diffing wall times of programs that
    loop the whole body R1 vs R2 times on-device (cancels the ~105ms axon
    RPC floor)."""
    in_maps, layout1, layout2, halo1, xt = _prep_all(inputs)
    walls = {}
    outs = {}
    for R in loop_reps:
        nc = _build_program(layout1, layout2, halo1, xt, bench_reps=R)
        results, times = _run_pjrt(nc, in_maps, reps=wall_reps)
        walls[R] = min(times)
        out = np.empty((N, 1), np.float32)
        for core in range(NCORES):
            out[core * PER:(core + 1) * PER, 0] = \
                results[core]["out"].reshape(M_OUT)[:PER]
        outs[R] = out
    R1, R2 = loop_reps
    per_iter = (walls[R2] - walls[R1]) / (R2 - R1)
    return per_iter, walls, outs


# revision 35
# speedup vs baseline: 1.0100x; 1.0100x over previous
"""Trainium2 Bass kernel for nn_DiffusionCNN (submanifold sparse 3x3x3 CNN).

Strategy (8-core SPMD, no collectives):
  - Shard the voxel dim N=200000 into 8 contiguous blocks of 25000 (voxels are
    sorted by linear grid index, so neighbor indices are within ~+-1800 rows).
    Halo compute is replicated instead of exchanged.
  - Only ~13% of the 27 neighbor offsets are valid (occupancy ~9.5%); the
    baseline gathered all 27 (mostly zero rows).  Here we gather ONLY valid
    neighbors, compacted per 512-row tile and grouped by offset k:
      * per-offset z-matmuls: z[j,:] = W_k^T x[nbr_j]  (PE, 32/64/128-wide
        slots at aligned PSUM partition offsets),
      * expansion matmuls with a 0/1 matrix A[j,i] = (seg[j]==i) built on
        the DVE via iota/is_equal, accumulated into the [128,512] output.
    The center offset (always valid, identity map) skips the gather: its
    term is a single matmul against a channel-major resident copy of x/h1.
  - Slot sizes per (tile,k) are the max over the 8 cores, so ONE compiled
    program serves all cores; per-core index/seg tables fill the slots.
  - Gathers stay on one SWDGE queue: concurrent gathers on different queues
    corrupt data on this stack (verified with a minimal repro).
  - All matmuls in bf16 with fp32 PSUM accumulation.

Host-side work is limited to sharding/marshalling: slicing inputs, remapping
neighbor indices to per-core compacted tables, packing weights, re-assembling
the output.
"""

import os
import numpy as np
import ml_dtypes

# ---------------------------------------------------------------- constants
N = 200000
PER = 25000
NCORES = 8
C = 128
K = 27
CENTER = 13
TEMB = 6
IN_CH = 7  # features(1) + sin/cos(6)

TILE = 512
NT1 = 56                 # h1 tiles per core
NT2 = 49                 # output tiles per core
M_H1 = NT1 * TILE        # 28672 h1 rows computed per core (incl. halo + pad)
M_OUT = NT2 * TILE       # 25088 output rows per core (25000 + pad)
NZ = 512                 # zero rows at the front of each gather table
H1T = NZ + M_H1          # h1 table rows
# NOTE: concurrent gathers on different SWDGE queues corrupt data on this
# stack (verified with a minimal repro) -- stay on one queue.
NQ = int(os.environ.get("KNQ", "1"))   # SWDGE queues declared
NQ_USE = int(os.environ.get("KNQ_USE", str(NQ)))  # queues actually used

_bf16 = ml_dtypes.bfloat16


# ------------------------------------------------------------------ layout
def _pack_tile_items(items):
    """items: list of (k, n) with exact n in [1,128].  Pack into 128-row PSUM
    chunks.  PSUM partition bases are limited to {0,32,64}: each chunk holds
    either one slot with n>64 (base 0), or up to two slots with n<=64 at
    bases 0 and 64.  Gather columns pack tightly (gcol assigned later).
    Returns list of chunks; each chunk is a list of (k, jb, n) slots."""
    big = [(k, n) for (k, n) in items if n > 64]
    mid = [(k, n) for (k, n) in items if 32 < n <= 64]
    small = [(k, n) for (k, n) in items if n <= 32]
    chunks = [[(k, 0, n)] for (k, n) in big]
    while len(mid) >= 2:
        a, b = mid.pop(), mid.pop()
        chunks.append([(a[0], 0, a[1]), (b[0], 64, b[1])])
    if mid:
        # one leftover mid: put it at 64, fill 0/32 with smalls if any
        a = mid.pop()
        ents = []
        if small:
            s = small.pop()
            ents.append((s[0], 0, s[1]))
        if small:
            s = small.pop()
            ents.append((s[0], 32, s[1]))
        ents.append((a[0], 64, a[1]))
        chunks.append(ents)
    while small:
        # up to three smalls per chunk at bases 0/32/64 (64-base may be <=64)
        ents = []
        for base in (0, 32, 64):
            if small:
                s = small.pop()
                ents.append((s[0], base, s[1]))
        chunks.append(ents)
    if not chunks:
        chunks.append([(-1, 0, 1)])
    return chunks


def _build_layout(counts, NT):
    """counts: [NCORES][NT][K] valid counts.  Returns per-tile chunk layouts
    shared by all cores (slot budget = exact max over cores)."""
    tiles = []
    for t in range(NT):
        items = []
        for k in range(K):
            if k == CENTER:
                continue
            B = max(counts[c][t][k] for c in range(NCORES))
            if B == 0:
                continue
            assert B <= 128, (t, k, B)
            items.append((k, B))
        tiles.append(_pack_tile_items(items))
    return tiles


def _layout_cols(chunks):
    """Assign packed gather-column offsets to each slot.  Returns
    (slots, mt): slots = [(k, chunk, jb, n, gcol)], mt = padded idx count."""
    slots = []
    col = 0
    for cix, ents in enumerate(chunks):
        for (k, jb, n) in ents:
            slots.append((k, cix, jb, n, col))
            col += n
    mt = (col + 127) // 128 * 128
    return slots, mt


# ------------------------------------------------------------- device program
def _build_program(layout1, layout2, halo1, xt, bench_reps=0):
    import concourse.mybir as mybir
    import concourse.tile as tile
    from concourse import bacc
    from concourse.masks import make_identity

    bf = mybir.dt.bfloat16
    f32 = mybir.dt.float32
    i16 = mybir.dt.int16
    i32 = mybir.dt.int32
    AF = mybir.ActivationFunctionType
    EQ = mybir.AluOpType.is_equal

    maxch = max(len(t) for t in (layout1 + layout2))
    cols1 = [_layout_cols(t) for t in layout1]
    cols2 = [_layout_cols(t) for t in layout2]
    mts1 = [mt for (_, mt) in cols1]
    mts2 = [mt for (_, mt) in cols2]
    maxm = max(mts1 + mts2)
    ilen1, ilen2 = sum(mts1) // 16, sum(mts2) // 16
    slen1 = sum(len(t) for t in layout1)
    slen2 = sum(len(t) for t in layout2)

    nc = bacc.Bacc("TRN2", target_bir_lowering=False, debug=False,
                   num_swdge_queues=NQ)

    x_tab = nc.dram_tensor("x_tab", [xt, C], bf, kind="ExternalInput")
    xcm = nc.dram_tensor("xcm", [8, M_H1], bf, kind="ExternalInput")
    i1 = nc.dram_tensor("i1", [128, ilen1], i16, kind="ExternalInput")
    i2 = nc.dram_tensor("i2", [128, ilen2], i16, kind="ExternalInput")
    s1 = nc.dram_tensor("s1", [128, slen1], f32, kind="ExternalInput")
    s2 = nc.dram_tensor("s2", [128, slen2], f32, kind="ExternalInput")
    w1 = nc.dram_tensor("w1", [C, K * C], bf, kind="ExternalInput")
    w1c = nc.dram_tensor("w1c", [8, C], bf, kind="ExternalInput")
    w2 = nc.dram_tensor("w2", [C, K * C], bf, kind="ExternalInput")
    w3 = nc.dram_tensor("w3", [C, C], bf, kind="ExternalInput")
    w4 = nc.dram_tensor("w4", [C, 16], bf, kind="ExternalInput")
    b1 = nc.dram_tensor("b1", [C, 1], f32, kind="ExternalInput")
    b2 = nc.dram_tensor("b2", [C, 1], f32, kind="ExternalInput")
    b3 = nc.dram_tensor("b3", [C, 1], f32, kind="ExternalInput")
    b4 = nc.dram_tensor("b4", [1, 1], f32, kind="ExternalInput")
    outd = nc.dram_tensor("out", [NT2, TILE], f32, kind="ExternalOutput")
    h1tab = nc.dram_tensor("h1_tab", [H1T, C], bf, kind="Internal")

    with tile.TileContext(nc) as tc:
        with (
            tc.tile_pool(name="const", bufs=1) as constp,
            tc.tile_pool(name="idx", bufs=3) as idxp,
            tc.tile_pool(name="seg", bufs=3) as segp,
            tc.tile_pool(name="gat", bufs=3) as gatp,
            tc.tile_pool(name="amat", bufs=2) as apool,
            tc.tile_pool(name="zsb", bufs=2) as zpool,
            tc.tile_pool(name="act", bufs=3) as actp,
            tc.tile_pool(name="stage", bufs=3) as stagep,
            tc.tile_pool(name="ost", bufs=3) as outp,
            tc.tile_pool(name="psz", bufs=2, space="PSUM") as pszp,
            tc.tile_pool(name="psacc", bufs=2, space="PSUM") as psacc,
            tc.tile_pool(name="pstr", bufs=2, space="PSUM") as pstr,
            tc.tile_pool(name="psout", bufs=2, space="PSUM") as psout,
        ):
            w1_sb = constp.tile([C, K * C], bf, tag="w1")
            nc.sync.dma_start(w1_sb[:], w1[:])
            w1c_sb = constp.tile([8, C], bf, tag="w1c")
            nc.sync.dma_start(w1c_sb[:], w1c[:])
            w2_sb = constp.tile([C, K * C], bf, tag="w2")
            nc.sync.dma_start(w2_sb[:], w2[:])
            w3_sb = constp.tile([C, C], bf, tag="w3")
            nc.sync.dma_start(w3_sb[:], w3[:])
            w4_sb = constp.tile([C, 16], bf, tag="w4")
            nc.sync.dma_start(w4_sb[:], w4[:])
            b1_sb = constp.tile([C, 1], f32, tag="b1")
            nc.sync.dma_start(b1_sb[:], b1[:])
            b2_sb = constp.tile([C, 1], f32, tag="b2")
            nc.sync.dma_start(b2_sb[:], b2[:])
            b3_sb = constp.tile([C, 1], f32, tag="b3")
            nc.sync.dma_start(b3_sb[:], b3[:])
            b4_sb = constp.tile([1, 1], f32, tag="b4")
            nc.sync.dma_start(b4_sb[:], b4[:])
            bz_sb = constp.tile([C, 1], f32, tag="bz")
            nc.vector.memset(bz_sb[:], 0.0)
            ident = constp.tile([C, C], bf, tag="ident")
            make_identity(nc, ident[:])
            iota_i = constp.tile([128, TILE], i32, tag="iotai")
            nc.gpsimd.iota(iota_i[:], pattern=[[1, TILE]],
                           channel_multiplier=0)
            iota_sb = constp.tile([128, TILE], f32, tag="iota")
            nc.vector.tensor_copy(iota_sb[:], iota_i[:])
            xcm_sb = constp.tile([8, M_H1], bf, tag="xcm")
            nc.sync.dma_start(xcm_sb[:], xcm[:])
            h1cm = constp.tile([C, M_H1], bf, tag="h1cm")
            zblk = constp.tile([C, NZ // 128 * C], bf, tag="zblk")
            nc.vector.memset(zblk[:], 0.0)
            nc.sync.dma_start(
                h1tab[0:NZ, :].rearrange("(p c) e -> p (c e)", c=NZ // 128),
                zblk[:])

            # init the z PSUM buffers so stale slot gaps are finite
            for zi in range(2):
                zps0 = pszp.tile([128, 512], f32, tag="zp")
                nc.vector.memset(zps0[:], 0.0)

            nchunk = TILE // 128
            ioff = {1: 0, 2: 0}
            soff = {1: 0, 2: 0}
            qctr = [0]
            reg_cache = {}

            def conv_tile(ph, t, chunks):
                nch = len(chunks)
                slots, mt = _layout_cols(chunks)
                iN, sN = (i1, s1) if ph == 1 else (i2, s2)
                tab = x_tab if ph == 1 else h1tab
                w_sb = w1_sb if ph == 1 else w2_sb
                io, so = ioff[ph], soff[ph]
                ioff[ph] += mt // 16
                soff[ph] += nch

                it = idxp.tile([128, maxm // 16], i16, tag="it")
                nc.sync.dma_start(it[:, :mt // 16],
                                  iN[:, io:io + mt // 16])
                sg = segp.tile([128, maxch], f32, tag="sg")
                nc.sync.dma_start(sg[:, :nch], sN[:, so:so + nch])
                g = gatp.tile([128, maxm], bf, tag="g")
                if mt not in reg_cache:
                    reg_cache[mt] = nc.gpsimd.to_reg(mt)
                nc.gpsimd.dma_gather(
                    out_ap=g[:, :mt].rearrange("p (o n) -> p o n", o=1),
                    in_ap=tab[:, :],
                    idxs_ap=it[:, :mt // 16],
                    num_idxs=mt,
                    num_idxs_reg=reg_cache[mt],
                    elem_size=C,
                    transpose=True,
                    single_packet=False,
                    queue_num=qctr[0] % NQ_USE,
                )
                qctr[0] += 1

                # expansion matrices A[j, i] = (seg[j] == i)
                A = apool.tile([128, maxch * TILE], bf, tag="A")
                for cix in range(nch):
                    nc.vector.tensor_scalar(
                        A[:, cix * TILE:(cix + 1) * TILE],
                        iota_sb[:], sg[:, cix:cix + 1], None, op0=EQ)

                # z[j, :] = W_k^T g[:, j], packed 4 chunks per PSUM bank
                zsb = zpool.tile([128, maxch * 128], bf, tag="z")
                for g0 in range(0, nch, 4):
                    gw = min(4, nch - g0)
                    zps = pszp.tile([128, 512], f32, tag="zp")
                    for (k, cix, jb, sz, gcol) in slots:
                        if not (g0 <= cix < g0 + gw):
                            continue
                        kk = 0 if k < 0 else k
                        nc.tensor.matmul(
                            zps[jb:jb + sz,
                                (cix - g0) * 128:(cix - g0 + 1) * 128],
                            lhsT=g[:, gcol:gcol + sz],
                            rhs=w_sb[:, kk * C:(kk + 1) * C],
                            start=True, stop=True)
                    nc.scalar.activation(
                        zsb[:, g0 * 128:(g0 + gw) * 128],
                        zps[:, :gw * 128], AF.Identity, bias=bz_sb[:, 0:1])

                acc = psacc.tile([C, TILE], f32, tag="acc")
                if ph == 1:
                    nc.tensor.matmul(
                        acc[:], lhsT=w1c_sb[:],
                        rhs=xcm_sb[0:8, t * TILE:(t + 1) * TILE],
                        start=True, stop=False)
                else:
                    nc.tensor.matmul(
                        acc[:], lhsT=w2_sb[:, CENTER * C:(CENTER + 1) * C],
                        rhs=h1cm[:, halo1 + t * TILE:halo1 + (t + 1) * TILE],
                        start=True, stop=False)
                for cix in range(nch):
                    nc.tensor.matmul(
                        acc[:], lhsT=zsb[:, cix * 128:(cix + 1) * 128],
                        rhs=A[:, cix * TILE:(cix + 1) * TILE],
                        start=False, stop=(cix == nch - 1))
                return acc

            def emit_body():
                ioff[1] = ioff[2] = soff[1] = soff[2] = 0
                # ------------ phase 1: h1 = silu(conv1(x)) ------------------
                for t in range(NT1):
                    acc = conv_tile(1, t, layout1[t])
                    nc.scalar.activation(
                        h1cm[:, t * TILE:(t + 1) * TILE], acc[:],
                        AF.Silu, bias=b1_sb[:, 0:1])
                    pt = pstr.tile([C, TILE], bf, tag="tr")
                    for cch in range(nchunk):
                        nc.tensor.matmul(
                            pt[:, 128 * cch:128 * (cch + 1)],
                            lhsT=h1cm[:, t * TILE + 128 * cch:
                                      t * TILE + 128 * (cch + 1)],
                            rhs=ident[:],
                            is_transpose=True,
                            start=(cch == 0),
                            stop=(cch == nchunk - 1),
                        )
                    st = stagep.tile([C, TILE], bf, tag="st")
                    nc.vector.tensor_copy(st[:], pt[:])
                    r0 = NZ + t * TILE
                    # physical row of logical in-tile row r is
                    # 4*(r%128) + r//128 (host compensates in conv2 indices)
                    nc.sync.dma_start(
                        h1tab[r0:r0 + TILE, :].rearrange(
                            "(p c) e -> p (c e)", c=nchunk),
                        st[:],
                    )

                # ------------ phase 2: conv2 + pointwise MLP ---------------
                for t in range(NT2):
                    acc = conv_tile(2, t, layout2[t])
                    h2 = actp.tile([C, TILE], bf, tag="h")
                    nc.scalar.activation(h2[:], acc[:], AF.Silu,
                                         bias=b2_sb[:, 0:1])
                    ps3 = psacc.tile([C, TILE], f32, tag="acc")
                    nc.tensor.matmul(ps3[:], lhsT=w3_sb[:], rhs=h2[:],
                                     start=True, stop=True)
                    h3 = actp.tile([C, TILE], bf, tag="h")
                    nc.scalar.activation(h3[:], ps3[:], AF.Silu,
                                         bias=b3_sb[:, 0:1])
                    ps4 = psout.tile([1, TILE], f32, tag="o")
                    nc.tensor.matmul(ps4[:], lhsT=w4_sb[:, 0:1], rhs=h3[:],
                                     start=True, stop=True)
                    ost = outp.tile([1, TILE], f32, tag="ost")
                    nc.scalar.activation(ost[0:1, :], ps4[:], AF.Identity,
                                         bias=b4_sb[0:1, 0:1])
                    nc.sync.dma_start(outd[t:t + 1, :], ost[0:1, :])

            if bench_reps > 0:
                with tc.For_i(0, bench_reps, 1):
                    emit_body()
            else:
                emit_body()

    nc.compile()
    return nc


# ------------------------------------------------------------------ host prep
def _sinusoidal(t):
    half = TEMB // 2
    freqs = (np.float32(2.0) ** np.arange(half, dtype=np.float32)) \
        * np.float32(np.pi)
    ang = t.astype(np.float32)[:, None] * freqs[None, :]
    return np.concatenate([np.sin(ang), np.cos(ang)], -1).astype(np.float32)


def _wrap_idx(flat):
    """[M] int32 -> [128, M/16] int16 (16-partition wrap, replicated x8)."""
    m = flat.shape[0]
    a = flat.reshape(m // 16, 16).T
    return np.tile(a, (8, 1)).astype(np.int16)


def _phys_h1_row(j):
    t = j // TILE
    r = j % TILE
    return t * TILE + 4 * (r % 128) + r // 128


def _core_valid(core, nidx, halo1):
    """Per-core valid lists for both phases.

    Returns (vals1, vals2, counts1, counts2, lo1) where valsP[t][k] =
    (pos_array, seg_array); pos is the global voxel row of the neighbor."""
    s = core * PER
    lo1 = s - halo1

    jl = np.arange(M_H1, dtype=np.int64)
    gj = lo1 + jl
    inb = (gj >= 0) & (gj < N)
    sub1 = np.full((K, M_H1), N, np.int32)
    sub1[:, inb] = nidx[:, gj[inb]]

    sub2 = np.full((K, M_OUT), N, np.int32)
    sub2[:, :PER] = nidx[:, s:s + PER]

    msk1 = np.stack([(sub1[12] < N), (sub1[14] < N)]).astype(_bf16)
    msk2 = np.stack([(sub2[12] < N), (sub2[14] < N)]).astype(_bf16)

    def per_tile(sub, NT):
        vals, counts = [], []
        for t in range(NT):
            blk = sub[:, t * TILE:(t + 1) * TILE]
            v = {}
            cnt = np.zeros(K, np.int32)
            for k in range(K):
                if k == CENTER:
                    continue
                segs = np.nonzero(blk[k] < N)[0]
                if len(segs) == 0:
                    continue
                v[k] = (blk[k][segs].astype(np.int64), segs.astype(np.int32))
                cnt[k] = len(segs)
            vals.append(v)
            counts.append(cnt)
        return vals, counts

    vals1, counts1 = per_tile(sub1, NT1)
    vals2, counts2 = per_tile(sub2, NT2)
    return vals1, vals2, counts1, counts2, lo1, msk1, msk2


def _fill_core(layout, vals, to_idx, rng):
    """Build the per-core gather-index and seg tables for one phase.

    Gather columns are packed (slot gcol offsets); seg rows are indexed by
    PSUM (chunk, partition)."""
    gidx_parts, gseg_parts = [], []
    for t, chunks in enumerate(layout):
        nch = len(chunks)
        slots, mt = _layout_cols(chunks)
        gi = rng.integers(0, NZ, size=mt).astype(np.int32)
        gs = np.full(nch * 128, -1, np.int32)
        for (k, cix, jb, sz, gcol) in slots:
            if k < 0 or k not in vals[t]:
                continue
            pos_arr, seg_arr = vals[t][k]
            idxs = to_idx(pos_arr)
            n = len(idxs)
            assert n <= sz, (t, k, n, sz)
            # ring r serves flat columns == r (mod 16); give each ring an
            # ascending address walk (measured ~13% faster than stride-16
            # sampling of the ascending run)
            perm = np.argsort((gcol + np.arange(n)) % 16, kind="stable")
            byp = np.argsort(idxs, kind="stable")
            gi[gcol + perm] = idxs[byp]
            gs[cix * 128 + jb + perm] = seg_arr[byp]
        gidx_parts.append(gi)
        gseg_parts.append(gs.reshape(nch, 128).T)
    gidx = np.concatenate(gidx_parts)
    gseg = np.concatenate(gseg_parts, axis=1)
    assert gidx.max() < 32768
    return _wrap_idx(gidx), gseg.astype(np.float32)


_PREP_CACHE = {}


def _prep_all(inputs):
    key = id(inputs.get("neighbor_idx"))
    features = np.asarray(inputs["features"], np.float32)
    t = np.asarray(inputs["t"])
    nidx = np.asarray(inputs["neighbor_idx"]).astype(np.int32)
    x_full = np.concatenate([features, _sinusoidal(t)], -1)

    # fixed h1 halo so the h1 window offset is core-independent
    halos, posts = [], []
    for core in range(NCORES):
        s, e = core * PER, (core + 1) * PER
        v = nidx[:, s:e]
        v = v[v < N]
        halos.append(max(0, s - int(v.min())))
        posts.append(max(0, int(v.max()) + 1 - e))
    halo1 = max(halos)
    post1 = max(posts)
    assert halo1 + PER + post1 <= M_H1, (halo1, post1)
    assert halo1 + M_OUT <= M_H1, halo1

    cores = []
    for core in range(NCORES):
        (vals1, vals2, counts1, counts2, lo1,
         msk1, msk2) = _core_valid(core, nidx, halo1)
        cores.append(dict(vals1=vals1, vals2=vals2, counts1=counts1,
                          counts2=counts2, lo1=lo1, msk1=msk1, msk2=msk2))

    layout1 = _build_layout([c["counts1"] for c in cores], NT1)
    layout2 = _build_layout([c["counts2"] for c in cores], NT2)

    # shared weights
    W1 = np.asarray(inputs["W1"], np.float32)
    W2 = np.asarray(inputs["W2"], np.float32)
    W1p = np.zeros((K, C, C), np.float32)
    W1p[:, :IN_CH, :] = W1
    w1d = np.ascontiguousarray(
        W1p.transpose(1, 0, 2).reshape(C, K * C)).astype(_bf16)
    w1cd = np.zeros((8, C), _bf16)
    w1cd[:IN_CH] = W1[CENTER].astype(_bf16)
    w2d = np.ascontiguousarray(
        W2.transpose(1, 0, 2).reshape(C, K * C)).astype(_bf16)
    w3d = np.ascontiguousarray(np.asarray(inputs["W3"], np.float32)) \
        .astype(_bf16)
    w4d = np.zeros((C, 16), _bf16)
    w4d[:, 0] = np.asarray(inputs["W4"], np.float32)[:, 0].astype(_bf16)
    shared = {
        "w1": w1d, "w1c": w1cd, "w2": w2d, "w3": w3d, "w4": w4d,
        "b1": np.asarray(inputs["b1"], np.float32).reshape(C, 1).copy(),
        "b2": np.asarray(inputs["b2"], np.float32).reshape(C, 1).copy(),
        "b3": np.asarray(inputs["b3"], np.float32).reshape(C, 1).copy(),
        "b4": np.asarray(inputs["b4"], np.float32).reshape(1, 1).copy(),
    }

    # x windows (per core), then a shared table size
    for cd in cores:
        allpos = [v[0] for vt in cd["vals1"] for v in vt.values()]
        allpos = np.concatenate(allpos) if allpos else np.array([0])
        cd["lo0"] = int(allpos.min())
        cd["hi0"] = int(allpos.max() + 1)
    xt = NZ + max(cd["hi0"] - cd["lo0"] for cd in cores)
    xt = (xt + 127) // 128 * 128
    assert xt < 32768, xt

    in_maps = []
    for core in range(NCORES):
        cd = cores[core]
        lo1 = cd["lo1"]
        lo0, hi0 = cd["lo0"], cd["hi0"]
        n0 = hi0 - lo0

        x_tab = np.zeros((xt, C), _bf16)
        x_tab[NZ:NZ + n0, :IN_CH] = x_full[lo0:hi0].astype(_bf16)

        xcm = np.zeros((8, M_H1), _bf16)
        jl = np.arange(M_H1, dtype=np.int64)
        gj = lo1 + jl
        inb = (gj >= 0) & (gj < N)
        xcm[:IN_CH, inb] = x_full[gj[inb]].astype(_bf16).T

        rng = np.random.default_rng(12345 + core)
        i1d, s1d = _fill_core(layout1, cd["vals1"],
                              lambda p: p - lo0 + NZ, rng)
        i2d, s2d = _fill_core(layout2, cd["vals2"],
                              lambda p: NZ + _phys_h1_row(p - lo1), rng)
        m = {"x_tab": x_tab, "xcm": xcm,
             "i1": i1d, "s1": s1d, "i2": i2d, "s2": s2d}
        m.update(shared)
        in_maps.append(m)

    return in_maps, layout1, layout2, halo1, xt


# ------------------------------------------------------------------ execution
def _run_pjrt(nc, in_maps, reps=0):
    """Execute the Bass program on the 8 axon-tunneled cores via PJRT."""
    import time as _time
    import jax
    from jax.sharding import Mesh, NamedSharding, PartitionSpec
    from jax.experimental.shard_map import shard_map
    import concourse.mybir as mybir
    from concourse import bass2jax

    bass2jax.install_neuronx_cc_hook()

    n_cores = len(in_maps)
    partition_name = (
        nc.partition_id_tensor.name if nc.partition_id_tensor else None
    )
    in_names, out_names, out_avals, zero_outs = [], [], [], []
    for alloc in nc.m.functions[0].allocations:
        if not isinstance(alloc, mybir.MemoryLocationSet):
            continue
        name = alloc.memorylocations[0].name
        if alloc.kind == "ExternalInput":
            if name != partition_name:
                in_names.append(name)
        elif alloc.kind == "ExternalOutput":
            shape = tuple(alloc.tensor_shape)
            dtype = mybir.dt.np(alloc.dtype)
            out_names.append(name)
            out_avals.append(jax.core.ShapedArray(shape, dtype))
            zero_outs.append(np.zeros(shape, dtype))
    n_params = len(in_names)
    n_outs = len(out_names)
    all_names = in_names + out_names
    if partition_name is not None:
        all_names = all_names + [partition_name]
    donate = tuple(range(n_params, n_params + n_outs))

    def _body(*args):
        operands = list(args)
        if partition_name is not None:
            operands.append(bass2jax.partition_id_tensor())
        outs = bass2jax._bass_exec_p.bind(
            *operands,
            out_avals=tuple(out_avals),
            in_names=tuple(all_names),
            out_names=tuple(out_names),
            lowering_input_output_aliases=(),
            sim_require_finite=True,
            sim_require_nnan=True,
            nc=nc,
        )
        return tuple(outs)

    devices = jax.devices()[:n_cores]
    mesh = Mesh(np.asarray(devices), ("core",))
    spec = PartitionSpec("core")
    sharded = jax.jit(
        shard_map(_body, mesh=mesh, in_specs=(spec,) * (n_params + n_outs),
                  out_specs=(spec,) * n_outs, check_rep=False),
        donate_argnums=donate,
        keep_unused=True,
    )
    concat_in = [
        np.concatenate([np.asarray(m[name]) for m in in_maps], axis=0)
        for name in in_names
    ]
    sh = NamedSharding(mesh, spec)
    inp_dev = [jax.device_put(a, sh) for a in concat_in]

    def _zeros():
        return [np.zeros((n_cores * z.shape[0], *z.shape[1:]), z.dtype)
                for z in zero_outs]

    out_arrs = sharded(*inp_dev, *_zeros())
    jax.block_until_ready(out_arrs)
    results = [
        {name: np.asarray(out_arrs[i]).reshape(n_cores, *out_avals[i].shape)[c]
         for i, name in enumerate(out_names)}
        for c in range(n_cores)
    ]

    times = []
    for _ in range(reps):
        zs = _zeros()
        t0 = _time.perf_counter()
        o = sharded(*inp_dev, *zs)
        jax.block_until_ready(o)
        times.append(_time.perf_counter() - t0)
    return results, times


_NC_CACHE = {}


def _run(inputs, reps=0):
    in_maps, layout1, layout2, halo1, xt = _prep_all(inputs)
    if "nc" not in _NC_CACHE:
        _NC_CACHE["nc"] = _build_program(layout1, layout2, halo1, xt)
    nc = _NC_CACHE["nc"]
    results, times = _run_pjrt(nc, in_maps, reps=reps)
    out = np.empty((N, 1), np.float32)
    for core in range(NCORES):
        out[core * PER:(core + 1) * PER, 0] = \
            results[core]["out"].reshape(M_OUT)[:PER]
    return out, times


def kernel(**inputs) -> np.ndarray:
    out, _ = _run(inputs, reps=0)
    return out


def bench(inputs, loop_reps=(1, 26), wall_reps=8):
    """Estimate on-device kernel time by diffing wall times of programs that
    loop the whole body R1 vs R2 times on-device (cancels the ~105ms axon
    RPC floor)."""
    in_maps, layout1, layout2, halo1, xt = _prep_all(inputs)
    walls = {}
    outs = {}
    for R in loop_reps:
        nc = _build_program(layout1, layout2, halo1, xt, bench_reps=R)
        results, times = _run_pjrt(nc, in_maps, reps=wall_reps)
        walls[R] = min(times)
        out = np.empty((N, 1), np.float32)
        for core in range(NCORES):
            out[core * PER:(core + 1) * PER, 0] = \
                results[core]["out"].reshape(M_OUT)[:PER]
        outs[R] = out
    R1, R2 = loop_reps
    per_iter = (walls[R2] - walls[R1]) / (R2 - R1)
    return per_iter, walls, outs
